# revision 1
# baseline (speedup 1.0000x reference)
"""AgentSelfAttention1d Trainium2 kernel.

Problem (per batch b of 8, one NeuronCore each):
    xt = x[b].T                       # [L=4096, D=512]
    q/k/v = xt @ W{q,k,v}.T + b       # [L, D]
    a  = AdaptiveAvgPool(q) -> [P=128, D]
    c  = softmax(a @ k.T, -1) @ v     # [P, D]
    r  = softmax(q @ a.T, -1) @ c     # [L, D]
    out[b] = r.T                      # [D, L]

Algebraic restructuring used here (everything channel-first on chip):
    apT[d,p]  = (Wq @ pool(x) / 32) + bq          "agent" tokens, [D, P]
    S1[p,l]   = sum_e H[e,p] x[e,l],  H[e,p] = sum_d Wk[d,e] apT[d,p]
                (k projection eliminated; bk drops out of softmax-1)
    E1        = exp(S1 - 10)                      free-axis softmax numerator
    M1[p,e]   = sum_l E1[p,l] x[e,l]   via PE-transposed E1 and x tiles
    c[p,d]    = (M1 @ Wv.T) / rowsum1[p] + bv     (v projection eliminated)
    S2T[p,l]  = sum_e G[e,p] x[e,l] + hq[p],  G from Wq like H,
                hq[p] = bq . a[p]                 (q projection eliminated)
    E2        = exp(S2T - 40);  colsum2[l] via PE ones-matmul
    out[d,l]  = (sum_p c[p,d] E2n[p,l]),  E2n = E2 * (1/colsum2) broadcast

All matmuls run in float32r (full-speed fp32 mode, ~1e-4 relative rounding).
Softmaxes use constant shifts instead of max-subtraction (logit absmax is
~21 / ~42 for this model; exp stays far inside fp32 range either way).
x is transposed on-chip with PE transpose-mode (saves 8 MB of HBM traffic
vs shipping x.T from the host).
"""

import numpy as np

import concourse.bass as bass
import concourse.mybir as mybir
import concourse.tile as tile
from concourse import bacc
from concourse.bass_utils import run_bass_kernel_spmd

F32R = mybir.dt.float32r
F32 = mybir.dt.float32

B, D, L, P = 8, 512, 4096, 128
KT = D // 128      # 4 contraction tiles of 128
NCH = L // 512     # 8 l-chunks of 512
NLT = L // 128     # 32 l-tiles of 128
SHIFT1 = 10.0      # constant logit shift, stage 1 (|S1| ~ 21)
SHIFT2 = 40.0      # constant logit shift, stage 2 (|S2| ~ 42)

_CACHE = {}


def build():
    nc = bacc.Bacc(target_bir_lowering=False, trn_type="TRN2")
    X = nc.dram_tensor("x", [D, L], F32R, kind="ExternalInput")
    WQT = nc.dram_tensor("WqT", [D, D], F32R, kind="ExternalInput")   # [e, d]
    WQN = nc.dram_tensor("Wqn", [D, D], F32R, kind="ExternalInput")   # [d, e]
    WKN = nc.dram_tensor("Wkn", [D, D], F32R, kind="ExternalInput")   # [d, e]
    WVT = nc.dram_tensor("WvT", [D, D], F32R, kind="ExternalInput")   # [e, d]
    BQC = nc.dram_tensor("bqc", [D, 2], F32R, kind="ExternalInput")   # [bq, 0]
    BQF = nc.dram_tensor("bqf", [D], F32, kind="ExternalInput")
    IDN = nc.dram_tensor("ident", [128, 128], F32R, kind="ExternalInput")
    ONE = nc.dram_tensor("ones128", [128, 128], F32R, kind="ExternalInput")
    BVF = nc.dram_tensor("bvf", [D], F32, kind="ExternalInput")
    OUT = nc.dram_tensor("out", [D, L], F32, kind="ExternalOutput")

    from contextlib import ExitStack
    with nc.allow_low_precision("float32r matmul operands"), \
         tile.TileContext(nc) as tc, ExitStack() as stack:
        sb = stack.enter_context(tc.tile_pool(name="sb", bufs=1))
        xtp = stack.enter_context(tc.tile_pool(name="xtp", bufs=31))
        e1p = stack.enter_context(tc.tile_pool(name="e1p", bufs=2))
        wnp = stack.enter_context(tc.tile_pool(name="wnp", bufs=1))
        pmp = stack.enter_context(tc.tile_pool(name="pmp", bufs=1))
        e1tp = stack.enter_context(tc.tile_pool(name="e1tp", bufs=1))
        outp = stack.enter_context(tc.tile_pool(name="outp", bufs=7))
        rbp = stack.enter_context(tc.tile_pool(name="rbp", bufs=1))
        # PSUM budget (8 banks): s:2 + tp:2 + small:1 + acc:1 + rt:2
        psS = stack.enter_context(tc.tile_pool(name="psS", bufs=2, space="PSUM"))
        psC = stack.enter_context(tc.tile_pool(name="psC", bufs=1, space="PSUM"))
        psT = stack.enter_context(tc.tile_pool(name="psT", bufs=2, space="PSUM"))
        psA = stack.enter_context(tc.tile_pool(name="psA", bufs=1, space="PSUM"))
        psR = stack.enter_context(tc.tile_pool(name="psR", bufs=2, space="PSUM"))

        # ---- ACT warmup: pull the activation-table load to t=0 ---------------
        warm = sb.tile([128, 1], F32)
        nc.vector.memset(warm, 0.0)
        nc.scalar.activation(out=warm, in_=warm,
                             func=mybir.ActivationFunctionType.Exp,
                             bias=warm, scale=1.0)

        # ---- x chunk-major + pooling + eager x-transposes --------------------
        # Chunk-major arrival means every x.T tile becomes transposable the
        # moment its chunk lands, so the PE fills the input-DMA window.
        ident = sb.tile([128, 128], F32R)
        nc.gpsimd.dma_start(out=ident, in_=IDN[:, :])
        x_sb = sb.tile([128, KT, L], F32R)
        xp = sb.tile([128, KT, P], F32R)
        xr = X.rearrange("(k p) l -> p k l", p=128)
        SEG = P // NCH
        xt_tiles = []
        alt = 0
        for ch in range(NCH):
            for h in range(2):
                hv = 2 * ch + h
                nc.sync.dma_start(
                    out=x_sb[:, :, bass.ts(hv, 256)], in_=xr[:, :, bass.ts(hv, 256)])
                nc.vector.reduce_sum(
                    out=xp[:, :, bass.ts(hv, SEG // 2)],
                    in_=x_sb[:, :, bass.ts(hv, 256)].rearrange(
                        "p k (s t) -> p k s t", t=L // P),
                    axis=mybir.AxisListType.X)
                for u in range(2):
                    j = 2 * hv + u
                    xps = psT.tile([128, 512], F32R, tag="tp")
                    for k in range(KT):
                        nc.tensor.transpose(xps[:, bass.ts(k, 128)],
                                            x_sb[:, k, bass.ts(j, 128)], ident)
                    xt_t = xtp.tile([128, D], F32R, tag="xt")
                    nc.scalar.copy(xt_t, xps)
                    alt += 1
                    xt_tiles.append(xt_t)

        # ---- startup-chain weights: apT needs wqt, S1 needs H needs wkn ------
        wqt = sb.tile([128, KT, D], F32R)
        wvt = sb.tile([128, KT, D], F32R)
        nc.sync.dma_start(out=wqt, in_=WQT.rearrange("(k p) e -> p k e", p=128))
        bqf = sb.tile([128, KT], F32)
        nc.sync.dma_start(out=bqf, in_=BQF.rearrange("(k p) -> p k", p=128))
        bqc = sb.tile([128, KT, 2], F32R)
        nc.sync.dma_start(out=bqc, in_=BQC.rearrange("(k p) t -> p k t", p=128))
        wkn = wnp.tile([128, KT, D], F32R, tag="wn")
        nc.sync.dma_start(out=wkn, in_=WKN.rearrange("(k p) e -> p k e", p=128))
        nc.sync.dma_start(out=wvt, in_=WVT.rearrange("(k p) e -> p k e", p=128))
        ones128 = sb.tile([128, 128], F32R)
        nc.sync.dma_start(out=ones128, in_=ONE[:, :])
        bvf = sb.tile([128, KT], F32)
        nc.sync.dma_start(out=bvf, in_=BVF.rearrange("(k p) -> p k", p=128))
        sh1 = sb.tile([128, 1], F32)
        nc.vector.memset(sh1, -SHIFT1)
        sh2 = sb.tile([128, 1], F32)
        nc.vector.memset(sh2, -SHIFT2)

        # ---- agent tokens, p-major (N=512 full-speed f32r), then transpose ---
        # ap_raw[p,d] = sum_e xp[e,p] WqT[e,d]; apT = ap_raw.T/32 + bq
        apps = psR.tile([128, D], F32, tag="rt")
        for k in range(KT):
            nc.tensor.matmul(apps, xp[:, k, :], wqt[:, k, :],
                             start=(k == 0), stop=(k == KT - 1))
        ap_sb = pmp.tile([128, D], F32R, tag="pm")
        nc.scalar.copy(ap_sb, apps)
        apt = sb.tile([128, KT, P], F32R)
        atps = psT.tile([128, 512], F32R, tag="tp")
        for u in range(KT):
            nc.tensor.transpose(atps[:, bass.ts(u, 128)],
                                ap_sb[:, bass.ts(u, 128)], ident)
        for u in range(KT):
            nc.scalar.activation(
                out=apt[:, u, :], in_=atps[:, bass.ts(u, 128)],
                func=mybir.ActivationFunctionType.Identity,
                bias=bqf[:, u:u + 1], scale=1.0 / (L // P))

        # ---- G/H p-major: h'[p,e] = sum_d apT[d,p] Wk[d,e], transpose back ---
        g_sb = sb.tile([128, KT, P], F32R)
        h_sb = sb.tile([128, KT, P], F32R)
        hps2 = psR.tile([128, D], F32, tag="rt")
        for k in range(KT):
            nc.tensor.matmul(hps2, apt[:, k, :], wkn[:, k, :],
                             start=(k == 0), stop=(k == KT - 1))
        hp_sb = pmp.tile([128, D], F32R, tag="pm")
        nc.scalar.copy(hp_sb, hps2)
        htps = psT.tile([128, 512], F32R, tag="tp")
        for u in range(KT):
            nc.tensor.transpose(htps[:, bass.ts(u, 128)],
                                hp_sb[:, bass.ts(u, 128)], ident)
        nc.vector.tensor_copy(h_sb, htps)
        wqn = wnp.tile([128, KT, D], F32R, tag="wn")
        nc.sync.dma_start(out=wqn, in_=WQN.rearrange("(k p) e -> p k e", p=128))
        gps2 = psR.tile([128, D], F32, tag="rt")
        for k in range(KT):
            nc.tensor.matmul(gps2, apt[:, k, :], wqn[:, k, :],
                             start=(k == 0), stop=(k == KT - 1))
        gp_sb = pmp.tile([128, D], F32R, tag="pm")
        nc.scalar.copy(gp_sb, gps2)
        gtps = psT.tile([128, 512], F32R, tag="tp")
        for u in range(KT):
            nc.tensor.transpose(gtps[:, bass.ts(u, 128)],
                                gp_sb[:, bass.ts(u, 128)], ident)
        nc.vector.tensor_copy(g_sb, gtps)
        hps = psR.tile([128, 2], F32, tag="rt")
        for k in range(KT):
            nc.tensor.matmul(hps, apt[:, k, :], bqc[:, k, :],
                             start=(k == 0), stop=(k == KT - 1))
        hq = sb.tile([128, 2], F32)
        nc.scalar.activation(out=hq, in_=hps,
                             func=mybir.ActivationFunctionType.Identity,
                             bias=sh2, scale=1.0)

        rs1 = sb.tile([128, NCH], F32)

        # ---- E1/x transposes + M1 = E1 @ x.T ---------------------------------
        # per group of 4 l-tiles: one [128,512] psum collects 4 E1 transposes;
        # per l-tile: one [128,512] psum collects 4 x transposes (-> x.T tile).
        e2 = sb.tile([128, NCH, 512], F32R)
        m1ps = psA.tile([128, D], F32, tag="acc")
        for a in range(NLT // 4):
            # stage-1 chunk a: scores + exp (accumulating row sums)
            ps1 = psS.tile([128, 512], F32, tag="s")
            for k in range(KT):
                nc.tensor.matmul(ps1, h_sb[:, k, :], x_sb[:, k, bass.ts(a, 512)],
                                 start=(k == 0), stop=(k == KT - 1))
            e1_t = e1p.tile([128, 512], F32R, tag="e1")
            nc.scalar.activation(out=e1_t, in_=ps1,
                                 func=mybir.ActivationFunctionType.Exp,
                                 bias=sh1, scale=1.0,
                                 accum_out=rs1[:, a:a + 1])
            # E1 transposes for this chunk + M1 accumulation (x.T prebuilt)
            eps = psT.tile([128, 512], F32R, tag="tp")
            for u in range(4):
                nc.tensor.transpose(eps[:, bass.ts(u, 128)],
                                    e1_t[:, bass.ts(u, 128)], ident)
            e1t_t = e1tp.tile([128, 4, 128], F32R, tag="e1t")
            nc.vector.tensor_copy(e1t_t, eps)
            for u in range(4):
                j = 4 * a + u
                nc.tensor.matmul(m1ps, e1t_t[:, u, :], xt_tiles[j],
                                 start=(j == 0), stop=(j == NLT - 1))
            # stage-2 chunk a first: its 5-stage chain overlaps the rest
            ps = psR.tile([128, 512], F32, tag="rt")
            for k in range(KT):
                nc.tensor.matmul(ps, g_sb[:, k, :], x_sb[:, k, bass.ts(a, 512)],
                                 start=(k == 0), stop=(k == KT - 1))
            nc.scalar.activation(out=e2[:, a, :], in_=ps,
                                 func=mybir.ActivationFunctionType.Exp,
                                 bias=hq[:, 0:1], scale=1.0)
            csps = psC.tile([128, 512], F32, tag="cs")
            nc.tensor.matmul(csps, ones128, e2[:, a, :], start=True, stop=True)
            rb = rbp.tile([128, 512], F32, tag="rb")
            nc.vector.reciprocal(rb, csps)
            nc.vector.tensor_mul(e2[:, a, :], e2[:, a, :], rb)
        m1 = sb.tile([128, D], F32R)
        nc.vector.tensor_copy(m1, m1ps)
        rsum1 = sb.tile([128, 1], F32)
        nc.vector.reduce_sum(out=rsum1, in_=rs1, axis=mybir.AxisListType.X)
        inv1 = sb.tile([128, 1], F32)
        nc.vector.reciprocal(inv1, rsum1)

        # ---- c = (M1 @ WvT)/rowsum1 + bv --------------------------------------
        m1t = sb.tile([128, KT, 128], F32R)
        mps = psT.tile([128, 512], F32R, tag="tp")
        for i in range(KT):
            nc.tensor.transpose(mps[:, bass.ts(i, 128)], m1[:, bass.ts(i, 128)],
                                ident)
        nc.vector.tensor_copy(m1t, mps)
        cps = psA.tile([128, D], F32, tag="acc")
        for i in range(KT):
            nc.tensor.matmul(cps, m1t[:, i, :], wvt[:, i, :],
                             start=(i == 0), stop=(i == KT - 1))
        # bv is NOT added to c here: stage-2 softmax columns sum to 1, so
        # bv^T @ E2n == bv broadcast; it is applied as a per-partition bias
        # in the output copies instead.
        c_sb = sb.tile([128, D], F32R)
        nc.scalar.activation(out=c_sb, in_=cps,
                             func=mybir.ActivationFunctionType.Copy,
                             bias=0.0, scale=inv1)

        # ---- output stream: out[d,l] = c.T @ E2n (DMA-bound) -----------------
        for ch in range(NCH):
            for d in range(KT):
                rps = psR.tile([128, 512], F32, tag="rt")
                nc.tensor.matmul(rps, c_sb[:, bass.ts(d, 128)], e2[:, ch, :],
                                 start=True, stop=True)
                o_t = outp.tile([128, 512], F32, tag="o")
                if (ch * KT + d) % 2 == 0:
                    nc.scalar.activation(
                        out=o_t, in_=rps,
                        func=mybir.ActivationFunctionType.Identity,
                        bias=bvf[:, d:d + 1], scale=1.0)
                else:
                    nc.vector.tensor_scalar_add(o_t, rps, bvf[:, d:d + 1])
                nc.sync.dma_start(
                    out=OUT[bass.ts(d, 128), bass.ts(ch, 512)], in_=o_t)


    nc.compile()
    return nc


def _host_inputs(x, Wq, bq, Wk, bk, Wv, bv):
    del bk  # stage-1 softmax is invariant to the k-projection bias
    common = {
        "WqT": np.ascontiguousarray(Wq.T),
        "Wqn": np.ascontiguousarray(Wq),
        "Wkn": np.ascontiguousarray(Wk),
        "WvT": np.ascontiguousarray(Wv.T),
        "bqc": np.ascontiguousarray(
            np.stack([bq, np.zeros_like(bq)], axis=1)),
        "bqf": np.ascontiguousarray(bq),
        "ident": np.eye(128, dtype=np.float32),
        "ones128": np.ones((128, 128), dtype=np.float32),
        "bvf": np.ascontiguousarray(bv),
    }
    maps = []
    for b in range(B):
        m = dict(common)
        m["x"] = np.ascontiguousarray(x[b])
        maps.append(m)
    return maps


def kernel(x, Wq, bq, Wk, bk, Wv, bv):
    x = np.asarray(x, dtype=np.float32)
    if "nc" not in _CACHE:
        _CACHE["nc"] = build()
    nc = _CACHE["nc"]
    in_maps = _host_inputs(x, np.asarray(Wq), np.asarray(bq), np.asarray(Wk),
                           np.asarray(bk), np.asarray(Wv), np.asarray(bv))
    res = run_bass_kernel_spmd(nc, in_maps, core_ids=list(range(B)))
    out = np.empty((B, D, L), dtype=np.float32)
    for b in range(B):
        out[b] = res.results[b]["out"]
    return out



# revision 9
# speedup vs baseline: 1.2328x; 1.2328x over previous
"""AgentSelfAttention1d Trainium2 kernel (v2).

Per batch b (one NeuronCore each):
    xt = x[b].T                       # [L=4096, D=512]
    q/k/v = xt @ W{q,k,v}.T + b       # [L, D]
    a  = AdaptiveAvgPool(q) -> [P=128, D]
    c  = softmax(a @ k.T, -1) @ v     # [P, D]
    r  = softmax(q @ a.T, -1) @ c     # [L, D]
    out[b] = r.T                      # [D, L]

Restructuring (all projections folded into host-precomputed weight
products; everything channel-first on chip):
    xp[c,p]   = seg-sum of x over 32-wide windows      (via tiny PE matmuls
                against a one-hot segment indicator, from the x.T tiles)
    H[e,p]    = MKs[c,e]^T-contract xp + vk[e],  MKs = (Wq^T Wk)/32,
                vk = Wk^T bq          (S1[p,l] = sum_e H[e,p] x[e,l])
    G[e,p]    = MQs-contract xp + vq[e],         MQs = (Wq^T Wq)/32
    hq[p]     = (xp^T (Wq^T bq))/32 + |bq|^2     (S2T[p,l] = G-part + hq)
    E1        = exp(S1 - 10)  bf16; rowsum via activation accumulator
    M1[p,e]   = E1 @ x.T   (E1 transposed by the DMA xbar engine)
    cbv[p,d]  = (M1 @ Wv^T) / rowsum1 + bv
    E2        = exp(S2T - 40) fp16 (unnormalized)
    out[l,d]  = (sum_p E2[p,l] cbv[p,d]) / colsum2[l]
                -- contraction over p needs NO transpose (p is already on
                partitions); colsum2 rides along as N=2 ones-matmuls and is
                applied as a per-partition scale on the output copies.
    Output written [L, D] fp16; host transposes/upcasts to [D, L] f32.

Softmaxes use constant logit shifts (|S1|~21, |S2|~42) instead of max
subtraction; exp stays in range (bf16 for E1, fp16 for E2).
"""

import numpy as np
import ml_dtypes

import concourse.bass as bass
import concourse.mybir as mybir
import concourse.tile as tile
from concourse import bacc
from concourse.bass_utils import run_bass_kernel_spmd

F32 = mybir.dt.float32
F16 = mybir.dt.float16
BF16 = mybir.dt.bfloat16

B, D, L, P = 8, 512, 4096, 128
KT = D // 128      # 4 contraction tiles of 128
NCH = L // 512     # 8 l-chunks of 512
NLT = L // 128     # 32 l-tiles of 128
SEG = L // P       # 32: pool segment length
SHIFT1 = 10.0
SHIFT2 = 40.0

_CACHE = {}


def build():
    nc = bacc.Bacc(target_bir_lowering=False, trn_type="TRN2")
    X = nc.dram_tensor("x", [D, L], F16, kind="ExternalInput")
    MKS = nc.dram_tensor("mks", [D, D], F16, kind="ExternalInput")   # (Wq^T Wk)/32 [c,e]
    MQS = nc.dram_tensor("mqs", [D, D], F16, kind="ExternalInput")   # (Wq^T Wq)/32 [c,e]
    WVT = nc.dram_tensor("wvt", [D, D], BF16, kind="ExternalInput")  # Wv^T [e,d]
    IDN = nc.dram_tensor("ident", [128, 128], F16, kind="ExternalInput")
    INDS = nc.dram_tensor("inds", [128, 4], BF16, kind="ExternalInput")  # l -> l//32 one-hot
    ONES2 = nc.dram_tensor("ones2", [128, 2], BF16, kind="ExternalInput")
    VK = nc.dram_tensor("vk", [D], F32, kind="ExternalInput")        # Wk^T bq
    VQ = nc.dram_tensor("vq", [D], F32, kind="ExternalInput")        # Wq^T bq
    VQ2 = nc.dram_tensor("vq2", [D, 2], F16, kind="ExternalInput")   # [(Wq^T bq)/32, 0]
    BCONST = nc.dram_tensor("bconst", [128], F32, kind="ExternalInput")  # |bq|^2-SHIFT2
    BVB = nc.dram_tensor("bvb", [128, D], BF16, kind="ExternalInput")     # bv bcast
    OUT = nc.dram_tensor("out", [L, D], F16, kind="ExternalOutput")

    from contextlib import ExitStack
    with nc.allow_low_precision("16-bit matmul operands"), \
         tile.TileContext(nc) as tc, ExitStack() as stack:
        sb = stack.enter_context(tc.tile_pool(name="sb", bufs=1))
        e1p = stack.enter_context(tc.tile_pool(name="e1p", bufs=3))
        e1tp = stack.enter_context(tc.tile_pool(name="e1tp", bufs=3))
        e2p = stack.enter_context(tc.tile_pool(name="e2p", bufs=3))
        outp = stack.enter_context(tc.tile_pool(name="outp", bufs=2))
        iv2p = stack.enter_context(tc.tile_pool(name="iv2p", bufs=2))
        # PSUM (8 banks): xT 2 + xp 1 + hg 1 | pass1: s1 2 + m1 1 |
        # pass2: s2 2 + out 3 + rs2 1
        psT = stack.enter_context(tc.tile_pool(name="psT", bufs=2, space="PSUM"))
        psS = stack.enter_context(tc.tile_pool(name="psS", bufs=2, space="PSUM"))
        psM = stack.enter_context(tc.tile_pool(name="psM", bufs=1, space="PSUM"))
        psO = stack.enter_context(tc.tile_pool(name="psO", bufs=2, space="PSUM"))
        psR = stack.enter_context(tc.tile_pool(name="psR", bufs=1, space="PSUM"))

        # ---- ACT table warmup ------------------------------------------------
        warm = sb.tile([128, 1], F32)
        nc.vector.memset(warm, 0.0)
        nc.scalar.activation(out=warm, in_=warm,
                             func=mybir.ActivationFunctionType.Exp,
                             bias=warm, scale=1.0)

        # ---- input DMAs (order = DMA_ENGINES order) --------------------------
        ident = sb.tile([128, 128], F16)
        nc.gpsimd.dma_start(out=ident, in_=IDN[:, :])        # SWDGE, off HWDGE
        inds = sb.tile([128, 4], BF16)
        nc.gpsimd.dma_start(out=inds, in_=INDS[:, :])
        x_sb = sb.tile([128, KT, L], F16)
        xr = X.rearrange("(k p) l -> p k l", p=128)
        for ch in range(NCH):
            nc.sync.dma_start(out=x_sb[:, :, bass.ts(ch, 512)],
                              in_=xr[:, :, bass.ts(ch, 512)])
        mks = sb.tile([128, KT, D], F16)
        nc.sync.dma_start(out=mks, in_=MKS.rearrange("(k p) e -> p k e", p=128))
        vk = sb.tile([128, KT], F32)
        nc.gpsimd.dma_start(out=vk, in_=VK.rearrange("(k p) -> p k", p=128))
        vq = sb.tile([128, KT], F32)
        nc.gpsimd.dma_start(out=vq, in_=VQ.rearrange("(k p) -> p k", p=128))
        vq2 = sb.tile([128, KT, 2], F16)
        nc.gpsimd.dma_start(out=vq2, in_=VQ2.rearrange("(k p) t -> p k t", p=128))
        bconst = sb.tile([128, 1], F32)
        nc.gpsimd.dma_start(out=bconst, in_=BCONST.rearrange("(p o) -> p o", o=1))
        ones2 = sb.tile([128, 2], BF16)
        nc.gpsimd.dma_start(out=ones2, in_=ONES2[:, :])
        bvb = sb.tile([128, D], BF16)
        nc.gpsimd.dma_start(out=bvb, in_=BVB[:, :])
        mqs = sb.tile([128, KT, D], F16)
        nc.sync.dma_start(out=mqs, in_=MQS.rearrange("(k p) e -> p k e", p=128))
        wvt = sb.tile([128, KT, D], BF16)
        nc.sync.dma_start(out=wvt, in_=WVT.rearrange("(k p) e -> p k e", p=128))
        sh1 = sb.tile([128, 1], F32)
        nc.vector.memset(sh1, -SHIFT1)

        # ---- x.T tiles (PE transpose) + pooling (tiny PE matmuls) ------------
        xt = sb.tile([128, NLT, D], BF16)
        xpps = psM.tile([128, KT, 128], F32, tag="m")
        cp_eng = 0
        for j in range(NLT):
            tp = psT.tile([128, 512], F16, tag="t")
            for k in range(KT):
                nc.tensor.transpose(tp[:, bass.ts(k, 128)],
                                    x_sb[:, k, bass.ts(j, 128)], ident)
            if cp_eng == 0:
                nc.scalar.copy(xt[:, j, :], tp)
            else:
                nc.vector.tensor_copy(xt[:, j, :], tp)
            cp_eng = (cp_eng + 1) % 2
            for t in range(KT):
                nc.tensor.matmul(xpps[:, t, 4 * j:4 * j + 4],
                                 xt[:, j, bass.ts(t, 128)], inds,
                                 start=True, stop=True)
        xp = sb.tile([128, KT, 128], F16)
        nc.scalar.copy(xp, xpps)

        # ---- H, G, hq --------------------------------------------------------
        h_sb = sb.tile([128, KT, 128], F16)
        hps = psR.tile([128, KT, 128], F32, tag="r")
        for et in range(KT):
            for ck in range(KT):
                nc.tensor.matmul(hps[:, et, :],
                                 mks[:, ck, bass.ts(et, 128)], xp[:, ck, :],
                                 start=(ck == 0), stop=(ck == KT - 1))
        for et in range(KT):
            nc.scalar.activation(out=h_sb[:, et, :], in_=hps[:, et, :],
                                 func=mybir.ActivationFunctionType.Identity,
                                 bias=vk[:, et:et + 1], scale=1.0)
        hqps = psR.tile([128, 2], F32, tag="r")
        for ck in range(KT):
            nc.tensor.matmul(hqps, xp[:, ck, :], vq2[:, ck, :],
                             start=(ck == 0), stop=(ck == KT - 1))
        hq = sb.tile([128, 1], F32)
        nc.scalar.activation(out=hq, in_=hqps[:, 0:1],
                             func=mybir.ActivationFunctionType.Identity,
                             bias=bconst, scale=1.0)
        g_sb = sb.tile([128, KT, 128], F16)
        gps = psR.tile([128, KT, 128], F32, tag="r")
        for et in range(KT):
            for ck in range(KT):
                nc.tensor.matmul(gps[:, et, :],
                                 mqs[:, ck, bass.ts(et, 128)], xp[:, ck, :],
                                 start=(ck == 0), stop=(ck == KT - 1))
        for et in range(KT):
            nc.scalar.activation(out=g_sb[:, et, :], in_=gps[:, et, :],
                                 func=mybir.ActivationFunctionType.Identity,
                                 bias=vq[:, et:et + 1], scale=1.0)

        # ---- pass 1: S1 -> E1 -> (xbar) E1T -> M1 ----------------------------
        rs1 = sb.tile([128, NCH], F32)
        m1ps = psM.tile([128, D], F32, tag="m")
        for a in range(NCH):
            s1 = psS.tile([128, 512], F32, tag="s")
            for k in range(KT):
                nc.tensor.matmul(s1, h_sb[:, k, :], x_sb[:, k, bass.ts(a, 512)],
                                 start=(k == 0), stop=(k == KT - 1))
            e1 = e1p.tile([128, 512], BF16, tag="e1")
            nc.scalar.activation(out=e1, in_=s1,
                                 func=mybir.ActivationFunctionType.Exp,
                                 bias=sh1, scale=1.0,
                                 accum_out=rs1[:, a:a + 1])
            e1t = e1tp.tile([128, 4, 128], BF16, tag="e1t")
            nc.sync.dma_start_transpose(e1t, e1)
            for u in range(4):
                j = 4 * a + u
                nc.tensor.matmul(m1ps, e1t[:, u, :], xt[:, j, :],
                                 start=(j == 0), stop=(j == NLT - 1))

        rsum1 = sb.tile([128, 1], F32)
        nc.vector.reduce_sum(out=rsum1, in_=rs1, axis=mybir.AxisListType.X)
        inv1 = sb.tile([128, 1], F32)
        nc.vector.reciprocal(inv1, rsum1)

        # ---- c = (M1 @ Wv^T)/rowsum1 + bv ------------------------------------
        identb = sb.tile([128, 128], BF16)
        nc.vector.tensor_copy(identb, ident)
        m1 = sb.tile([128, D], BF16)
        nc.scalar.copy(m1, m1ps)
        mtp = psT.tile([128, 512], BF16, tag="t")
        for i in range(KT):
            nc.tensor.transpose(mtp[:, bass.ts(i, 128)], m1[:, bass.ts(i, 128)],
                                identb)
        m1t = sb.tile([128, KT, 128], BF16)
        nc.scalar.copy(m1t, mtp)
        cps = psR.tile([128, D], F32, tag="r")
        for i in range(KT):
            nc.tensor.matmul(cps, m1t[:, i, :], wvt[:, i, :],
                             start=(i == 0), stop=(i == KT - 1))
        cbv = sb.tile([128, D], BF16)
        nc.vector.scalar_tensor_tensor(out=cbv, in0=cps, scalar=inv1, in1=bvb,
                                       op0=mybir.AluOpType.mult,
                                       op1=mybir.AluOpType.add)

        # ---- pass 2: S2 -> E2 -> out = (E2^T cbv) * inv2 ---------------------
        # ident (fp16) reused as transpose stationary; e2 slices are the
        # stationary lhs of the output matmuls (contraction over p needs no
        # transpose), ones2 column rides along for colsum2.
        or_ = OUT.rearrange("(c j p) d -> c p j d", j=4, p=128)
        cp2 = 0
        for a in range(NCH):
            s2 = psS.tile([128, 512], F32, tag="s")
            for k in range(KT):
                nc.tensor.matmul(s2, g_sb[:, k, :], x_sb[:, k, bass.ts(a, 512)],
                                 start=(k == 0), stop=(k == KT - 1))
            e2 = e2p.tile([128, 512], BF16, tag="e2")
            nc.scalar.activation(out=e2, in_=s2,
                                 func=mybir.ActivationFunctionType.Exp,
                                 bias=hq, scale=1.0)
            rsps = psR.tile([128, 4, 2], F32, tag="r")
            for u in range(4):
                nc.tensor.matmul(rsps[:, u, :], e2[:, bass.ts(u, 128)], ones2,
                                 start=True, stop=True)
            inv2 = iv2p.tile([128, 4], F32, tag="iv2")
            nc.vector.reciprocal(inv2, rsps[:, :, 0])
            o_sb = outp.tile([128, 4, D], F16, tag="o")
            for u in range(4):
                ops = psO.tile([128, D], F32, tag="o")
                nc.tensor.matmul(ops, e2[:, bass.ts(u, 128)], cbv,
                                 start=True, stop=True)
                if cp2 == 0:
                    nc.scalar.activation(
                        out=o_sb[:, u, :], in_=ops,
                        func=mybir.ActivationFunctionType.Identity,
                        bias=0.0, scale=inv2[:, u:u + 1])
                else:
                    nc.vector.tensor_scalar_mul(o_sb[:, u, :], ops,
                                                inv2[:, u:u + 1])
                cp2 = (cp2 + 1) % 2
            nc.sync.dma_start(out=or_[a], in_=o_sb)

    nc.compile()
    return nc


def _host_inputs(x, Wq, bq, Wk, bk, Wv, bv):
    del bk  # stage-1 softmax is invariant to the k-projection bias
    Wq = np.asarray(Wq, dtype=np.float32)
    Wk = np.asarray(Wk, dtype=np.float32)
    Wv = np.asarray(Wv, dtype=np.float32)
    bq = np.asarray(bq, dtype=np.float32)
    bv = np.asarray(bv, dtype=np.float32)
    bf16 = ml_dtypes.bfloat16
    inds = np.zeros((128, 4), dtype=np.float32)
    inds[np.arange(128), np.arange(128) // SEG] = 1.0
    ones2 = np.zeros((128, 2), dtype=np.float32)
    ones2[:, 0] = 1.0
    common = {
        "mks": ((Wq.T @ Wk) / SEG).astype(np.float16),
        "mqs": ((Wq.T @ Wq) / SEG).astype(np.float16),
        "wvt": np.ascontiguousarray(Wv.T).astype(bf16),
        "ident": np.eye(128, dtype=np.float16),
        "inds": inds.astype(bf16),
        "ones2": ones2.astype(bf16),
        "vk": (Wk.T @ bq).astype(np.float32),
        "vq": (Wq.T @ bq).astype(np.float32),
        "vq2": np.stack([(Wq.T @ bq) / SEG, np.zeros(D, np.float32)],
                        axis=1).astype(np.float16),
        "bconst": np.full(128, float(bq @ bq) - SHIFT2, dtype=np.float32),
        "bvb": np.tile(bv[None, :], (128, 1)).astype(bf16),
    }
    maps = []
    for b in range(B):
        m = dict(common)
        m["x"] = np.ascontiguousarray(x[b]).astype(np.float16)
        maps.append(m)
    return maps


def kernel(x, Wq, bq, Wk, bk, Wv, bv):
    x = np.asarray(x, dtype=np.float32)
    if "nc" not in _CACHE:
        _CACHE["nc"] = build()
    nc = _CACHE["nc"]
    in_maps = _host_inputs(x, Wq, bq, Wk, bk, Wv, bv)
    res = run_bass_kernel_spmd(nc, in_maps, core_ids=list(range(B)))
    out = np.empty((B, D, L), dtype=np.float32)
    for b in range(B):
        out[b] = np.asarray(res.results[b]["out"]).astype(np.float32).T
    return out


# revision 14
# speedup vs baseline: 1.3361x; 1.0838x over previous
"""AgentSelfAttention1d Trainium2 kernel (v2).

Per batch b (one NeuronCore each):
    xt = x[b].T                       # [L=4096, D=512]
    q/k/v = xt @ W{q,k,v}.T + b       # [L, D]
    a  = AdaptiveAvgPool(q) -> [P=128, D]
    c  = softmax(a @ k.T, -1) @ v     # [P, D]
    r  = softmax(q @ a.T, -1) @ c     # [L, D]
    out[b] = r.T                      # [D, L]

Restructuring (all projections folded into host-precomputed weight
products; everything channel-first on chip):
    xp[c,p]   = seg-sum of x over 32-wide windows      (via tiny PE matmuls
                against a one-hot segment indicator, from the x.T tiles)
    H[e,p]    = MKs[c,e]^T-contract xp + vk[e],  MKs = (Wq^T Wk)/32,
                vk = Wk^T bq          (S1[p,l] = sum_e H[e,p] x[e,l])
    G[e,p]    = MQs-contract xp + vq[e],         MQs = (Wq^T Wq)/32
    hq[p]     = (xp^T (Wq^T bq))/32 + |bq|^2     (S2T[p,l] = G-part + hq)
    E1        = exp(S1 - 10)  bf16; rowsum via activation accumulator
    M1[p,e]   = E1 @ x.T   (E1 transposed by the DMA xbar engine)
    cbv[p,d]  = (M1 @ Wv^T) / rowsum1 + bv
    E2        = exp(S2T - 40) fp16 (unnormalized)
    out[l,d]  = (sum_p E2[p,l] cbv[p,d]) / colsum2[l]
                -- contraction over p needs NO transpose (p is already on
                partitions); colsum2 rides along as N=2 ones-matmuls and is
                applied as a per-partition scale on the output copies.
    Output written [L, D] fp16; host transposes/upcasts to [D, L] f32.

Softmaxes use constant logit shifts (|S1|~21, |S2|~42) instead of max
subtraction; exp stays in range (bf16 for E1, fp16 for E2).
"""

import numpy as np
import ml_dtypes

import concourse.bass as bass
import concourse.mybir as mybir
import concourse.tile as tile
from concourse import bacc
from concourse.bass_utils import run_bass_kernel_spmd

F32 = mybir.dt.float32
F16 = mybir.dt.float16
BF16 = mybir.dt.bfloat16

B, D, L, P = 8, 512, 4096, 128
KT = D // 128      # 4 contraction tiles of 128
NCH = L // 512     # 8 l-chunks of 512
NLT = L // 128     # 32 l-tiles of 128
SEG = L // P       # 32: pool segment length
SHIFT1 = 10.0
SHIFT2 = 40.0

_CACHE = {}


def build():
    nc = bacc.Bacc(target_bir_lowering=False, trn_type="TRN2")
    X = nc.dram_tensor("x", [D, L], F16, kind="ExternalInput")
    MKS = nc.dram_tensor("mks", [D, D], F16, kind="ExternalInput")   # (Wq^T Wk)/32 [c,e]
    MQS = nc.dram_tensor("mqs", [D, D], F16, kind="ExternalInput")   # (Wq^T Wq)/32 [c,e]
    WVT = nc.dram_tensor("wvt", [D, D], BF16, kind="ExternalInput")  # Wv^T [e,d]
    IDN = nc.dram_tensor("ident", [128, 128], F16, kind="ExternalInput")
    INDS = nc.dram_tensor("inds", [128, 4], BF16, kind="ExternalInput")  # l -> l//32 one-hot
    ONES2 = nc.dram_tensor("ones2", [128, 2], BF16, kind="ExternalInput")
    VK = nc.dram_tensor("vk", [D], F32, kind="ExternalInput")        # Wk^T bq
    VQ = nc.dram_tensor("vq", [D], F32, kind="ExternalInput")        # Wq^T bq
    VQ2 = nc.dram_tensor("vq2", [D, 2], F16, kind="ExternalInput")   # [(Wq^T bq)/32, 0]
    BCONST = nc.dram_tensor("bconst", [128], F32, kind="ExternalInput")  # |bq|^2-SHIFT2
    BVB = nc.dram_tensor("bvb", [128, D], BF16, kind="ExternalInput")     # bv bcast
    OUT = nc.dram_tensor("out", [L, D], F16, kind="ExternalOutput")

    from contextlib import ExitStack
    with nc.allow_low_precision("16-bit matmul operands"), \
         tile.TileContext(nc) as tc, ExitStack() as stack:
        sb = stack.enter_context(tc.tile_pool(name="sb", bufs=1))
        e1p = stack.enter_context(tc.tile_pool(name="e1p", bufs=3))
        e1tp = stack.enter_context(tc.tile_pool(name="e1tp", bufs=3))
        e2p = stack.enter_context(tc.tile_pool(name="e2p", bufs=3))
        outp = stack.enter_context(tc.tile_pool(name="outp", bufs=2))
        iv2p = stack.enter_context(tc.tile_pool(name="iv2p", bufs=2))
        # PSUM (8 banks): xT 2 + xp 1 + hg 1 | pass1: s1 2 + m1 1 |
        # pass2: s2 2 + out 3 + rs2 1
        psT = stack.enter_context(tc.tile_pool(name="psT", bufs=2, space="PSUM"))
        psS = stack.enter_context(tc.tile_pool(name="psS", bufs=2, space="PSUM"))
        psM = stack.enter_context(tc.tile_pool(name="psM", bufs=1, space="PSUM"))
        psO = stack.enter_context(tc.tile_pool(name="psO", bufs=2, space="PSUM"))
        psR = stack.enter_context(tc.tile_pool(name="psR", bufs=1, space="PSUM"))

        # ---- ACT table warmup ------------------------------------------------
        warm = sb.tile([128, 1], F32)
        nc.vector.memset(warm, 0.0)
        nc.scalar.activation(out=warm, in_=warm,
                             func=mybir.ActivationFunctionType.Exp,
                             bias=warm, scale=1.0)

        # ---- input DMAs (order = DMA_ENGINES order) --------------------------
        ident = sb.tile([128, 128], F16)
        nc.gpsimd.dma_start(out=ident, in_=IDN[:, :])        # SWDGE, off HWDGE
        inds = sb.tile([128, 4], BF16)
        nc.gpsimd.dma_start(out=inds, in_=INDS[:, :])
        x_sb = sb.tile([128, KT, L], F16)
        xr = X.rearrange("(k p) l -> p k l", p=128)
        for ch in range(NCH):
            nc.sync.dma_start(out=x_sb[:, :, bass.ts(ch, 512)],
                              in_=xr[:, :, bass.ts(ch, 512)])
        mks = sb.tile([128, KT, D], F16)
        nc.sync.dma_start(out=mks, in_=MKS.rearrange("(k p) e -> p k e", p=128))
        vk = sb.tile([128, KT], F32)
        nc.gpsimd.dma_start(out=vk, in_=VK.rearrange("(k p) -> p k", p=128))
        vq = sb.tile([128, KT], F32)
        nc.gpsimd.dma_start(out=vq, in_=VQ.rearrange("(k p) -> p k", p=128))
        vq2 = sb.tile([128, KT, 2], F16)
        nc.gpsimd.dma_start(out=vq2, in_=VQ2.rearrange("(k p) t -> p k t", p=128))
        bconst = sb.tile([128, 1], F32)
        nc.gpsimd.dma_start(out=bconst, in_=BCONST.rearrange("(p o) -> p o", o=1))
        ones2 = sb.tile([128, 2], BF16)
        nc.gpsimd.dma_start(out=ones2, in_=ONES2[:, :])
        bvb = sb.tile([128, D], BF16)
        nc.gpsimd.dma_start(out=bvb, in_=BVB[:, :])
        mqs = sb.tile([128, KT, D], F16)
        nc.sync.dma_start(out=mqs, in_=MQS.rearrange("(k p) e -> p k e", p=128))
        wvt = sb.tile([128, KT, D], BF16)
        nc.sync.dma_start(out=wvt, in_=WVT.rearrange("(k p) e -> p k e", p=128))
        sh1 = sb.tile([128, 1], F32)
        nc.vector.memset(sh1, -SHIFT1)

        # ---- x.T tiles (PE transpose) + pooling (tiny PE matmuls) ------------
        xt = sb.tile([128, NLT, D], BF16)
        xpps = psM.tile([128, KT, 128], F32, tag="m")
        for jp in range(NLT // 2):
            tp = psT.tile([128, 2, 512], F16, tag="t")
            for h in range(2):
                j = 2 * jp + h
                for k in range(KT):
                    nc.tensor.transpose(tp[:, h, bass.ts(k, 128)],
                                        x_sb[:, k, bass.ts(j, 128)], ident)
            if jp % 3 == 0:
                nc.scalar.copy(xt[:, 2 * jp:2 * jp + 2, :], tp)
            else:
                nc.vector.tensor_copy(xt[:, 2 * jp:2 * jp + 2, :], tp)
            for h in range(2):
                j = 2 * jp + h
                for t in range(KT):
                    nc.tensor.matmul(xpps[:, t, 4 * j:4 * j + 4],
                                     xt[:, j, bass.ts(t, 128)], inds,
                                     start=True, stop=True)
        xp = sb.tile([128, KT, 128], F16)
        nc.vector.tensor_copy(xp, xpps)

        # ---- H, G, hq (copies on DVE to keep Act free) -----------------------
        h_sb = sb.tile([128, KT, 128], F16)
        hps = psR.tile([128, KT, 128], F32, tag="r")
        for et in range(KT):
            for ck in range(KT):
                nc.tensor.matmul(hps[:, et, :],
                                 mks[:, ck, bass.ts(et, 128)], xp[:, ck, :],
                                 start=(ck == 0), stop=(ck == KT - 1))
        for et in range(KT):
            nc.vector.tensor_scalar_add(h_sb[:, et, :], hps[:, et, :],
                                        vk[:, et:et + 1])
        hqps = psR.tile([128, 2], F32, tag="r")
        for ck in range(KT):
            nc.tensor.matmul(hqps, xp[:, ck, :], vq2[:, ck, :],
                             start=(ck == 0), stop=(ck == KT - 1))
        hq = sb.tile([128, 1], F32)
        nc.vector.tensor_scalar_add(hq, hqps[:, 0:1], bconst)
        g_sb = sb.tile([128, KT, 128], F16)
        gps = psR.tile([128, KT, 128], F32, tag="r")
        for et in range(KT):
            for ck in range(KT):
                nc.tensor.matmul(gps[:, et, :],
                                 mqs[:, ck, bass.ts(et, 128)], xp[:, ck, :],
                                 start=(ck == 0), stop=(ck == KT - 1))
        for et in range(KT):
            nc.vector.tensor_scalar_add(g_sb[:, et, :], gps[:, et, :],
                                        vq[:, et:et + 1])

        # ---- pass 1: S1 -> E1 -> (xbar) E1T -> M1, software-pipelined --------
        # M1 for chunk a is issued after S1 for chunk a+2, so the PE never
        # stalls on the exp + xbar-transpose round trip.
        rs1 = sb.tile([128, NCH], F32)
        m1ps = psM.tile([128, D], F32, tag="m")
        e1ts = []

        def s1_stage(a):
            s1 = psS.tile([128, 512], F32, tag="s")
            for k in range(KT):
                nc.tensor.matmul(s1, h_sb[:, k, :], x_sb[:, k, bass.ts(a, 512)],
                                 start=(k == 0), stop=(k == KT - 1))
            e1 = e1p.tile([128, 512], BF16, tag="e1")
            nc.scalar.activation(out=e1, in_=s1,
                                 func=mybir.ActivationFunctionType.Exp,
                                 bias=sh1, scale=1.0,
                                 accum_out=rs1[:, a:a + 1])
            e1t = e1tp.tile([128, 4, 128], BF16, tag="e1t")
            nc.sync.dma_start_transpose(e1t, e1)
            e1ts.append(e1t)

        def m1_stage(a):
            e1t = e1ts[a]
            for u in range(4):
                j = 4 * a + u
                nc.tensor.matmul(m1ps, e1t[:, u, :], xt[:, j, :],
                                 start=(j == 0), stop=(j == NLT - 1))

        s1_stage(0)
        s1_stage(1)
        for a in range(NCH):
            if a + 2 < NCH:
                s1_stage(a + 2)
            m1_stage(a)

        rsum1 = sb.tile([128, 1], F32)
        nc.vector.reduce_sum(out=rsum1, in_=rs1, axis=mybir.AxisListType.X)
        inv1 = sb.tile([128, 1], F32)
        nc.vector.reciprocal(inv1, rsum1)

        # ---- c = (M1 @ Wv^T)/rowsum1 + bv ------------------------------------
        identb = sb.tile([128, 128], BF16)
        nc.vector.tensor_copy(identb, ident)
        m1 = sb.tile([128, D], BF16)
        nc.scalar.copy(m1, m1ps)
        mtp = psT.tile([128, 512], BF16, tag="t")
        for i in range(KT):
            nc.tensor.transpose(mtp[:, bass.ts(i, 128)], m1[:, bass.ts(i, 128)],
                                identb)
        m1t = sb.tile([128, KT, 128], BF16)
        nc.scalar.copy(m1t, mtp)
        cps = psR.tile([128, D], F32, tag="r")
        for i in range(KT):
            nc.tensor.matmul(cps, m1t[:, i, :], wvt[:, i, :],
                             start=(i == 0), stop=(i == KT - 1))
        cbv = sb.tile([128, D], BF16)
        nc.vector.scalar_tensor_tensor(out=cbv, in0=cps, scalar=inv1, in1=bvb,
                                       op0=mybir.AluOpType.mult,
                                       op1=mybir.AluOpType.add)

        # ---- pass 2: S2 -> E2 -> out = (E2^T cbv) * inv2 ---------------------
        # ident (fp16) reused as transpose stationary; e2 slices are the
        # stationary lhs of the output matmuls (contraction over p needs no
        # transpose), ones2 column rides along for colsum2.
        or_ = OUT.rearrange("(c j p) d -> c p j d", j=4, p=128)
        e2s = []

        def s2_stage(a):
            s2 = psS.tile([128, 512], F32, tag="s")
            for k in range(KT):
                nc.tensor.matmul(s2, g_sb[:, k, :], x_sb[:, k, bass.ts(a, 512)],
                                 start=(k == 0), stop=(k == KT - 1))
            e2 = e2p.tile([128, 512], BF16, tag="e2")
            nc.scalar.activation(out=e2, in_=s2,
                                 func=mybir.ActivationFunctionType.Exp,
                                 bias=hq, scale=1.0)
            e2s.append(e2)

        def out_stage(a):
            e2 = e2s[a]
            rsps = psR.tile([128, 4, 2], F32, tag="r")
            for u in range(4):
                nc.tensor.matmul(rsps[:, u, :], e2[:, bass.ts(u, 128)], ones2,
                                 start=True, stop=True)
            inv2 = iv2p.tile([128, 4], F32, tag="iv2")
            nc.vector.reciprocal(inv2, rsps[:, :, 0])
            o_sb = outp.tile([128, 4, D], F16, tag="o")
            for u in range(4):
                ops = psO.tile([128, D], F32, tag="o")
                nc.tensor.matmul(ops, e2[:, bass.ts(u, 128)], cbv,
                                 start=True, stop=True)
                if u % 2 == 0:
                    nc.scalar.activation(
                        out=o_sb[:, u, :], in_=ops,
                        func=mybir.ActivationFunctionType.Identity,
                        bias=0.0, scale=inv2[:, u:u + 1])
                else:
                    nc.vector.tensor_scalar_mul(o_sb[:, u, :], ops,
                                                inv2[:, u:u + 1])
            nc.sync.dma_start(out=or_[a], in_=o_sb)

        s2_stage(0)
        s2_stage(1)
        for a in range(NCH):
            if a + 2 < NCH:
                s2_stage(a + 2)
            out_stage(a)

    nc.compile()
    return nc


def _host_inputs(x, Wq, bq, Wk, bk, Wv, bv):
    del bk  # stage-1 softmax is invariant to the k-projection bias
    Wq = np.asarray(Wq, dtype=np.float32)
    Wk = np.asarray(Wk, dtype=np.float32)
    Wv = np.asarray(Wv, dtype=np.float32)
    bq = np.asarray(bq, dtype=np.float32)
    bv = np.asarray(bv, dtype=np.float32)
    bf16 = ml_dtypes.bfloat16
    inds = np.zeros((128, 4), dtype=np.float32)
    inds[np.arange(128), np.arange(128) // SEG] = 1.0
    ones2 = np.zeros((128, 2), dtype=np.float32)
    ones2[:, 0] = 1.0
    common = {
        "mks": ((Wq.T @ Wk) / SEG).astype(np.float16),
        "mqs": ((Wq.T @ Wq) / SEG).astype(np.float16),
        "wvt": np.ascontiguousarray(Wv.T).astype(bf16),
        "ident": np.eye(128, dtype=np.float16),
        "inds": inds.astype(bf16),
        "ones2": ones2.astype(bf16),
        "vk": (Wk.T @ bq).astype(np.float32),
        "vq": (Wq.T @ bq).astype(np.float32),
        "vq2": np.stack([(Wq.T @ bq) / SEG, np.zeros(D, np.float32)],
                        axis=1).astype(np.float16),
        "bconst": np.full(128, float(bq @ bq) - SHIFT2, dtype=np.float32),
        "bvb": np.tile(bv[None, :], (128, 1)).astype(bf16),
    }
    maps = []
    for b in range(B):
        m = dict(common)
        m["x"] = np.ascontiguousarray(x[b]).astype(np.float16)
        maps.append(m)
    return maps


def kernel(x, Wq, bq, Wk, bk, Wv, bv):
    x = np.asarray(x, dtype=np.float32)
    if "nc" not in _CACHE:
        _CACHE["nc"] = build()
    nc = _CACHE["nc"]
    in_maps = _host_inputs(x, Wq, bq, Wk, bk, Wv, bv)
    res = run_bass_kernel_spmd(nc, in_maps, core_ids=list(range(B)))
    out = np.empty((B, D, L), dtype=np.float32)
    for b in range(B):
        out[b] = np.asarray(res.results[b]["out"]).astype(np.float32).T
    return out


# revision 18
# speedup vs baseline: 1.3616x; 1.0191x over previous
"""AgentSelfAttention1d Trainium2 kernel (v2).

Per batch b (one NeuronCore each):
    xt = x[b].T                       # [L=4096, D=512]
    q/k/v = xt @ W{q,k,v}.T + b       # [L, D]
    a  = AdaptiveAvgPool(q) -> [P=128, D]
    c  = softmax(a @ k.T, -1) @ v     # [P, D]
    r  = softmax(q @ a.T, -1) @ c     # [L, D]
    out[b] = r.T                      # [D, L]

Restructuring (all projections folded into host-precomputed weight
products; everything channel-first on chip):
    xp[c,p]   = seg-sum of x over 32-wide windows      (via tiny PE matmuls
                against a one-hot segment indicator, from the x.T tiles)
    H[e,p]    = MKs[c,e]^T-contract xp + vk[e],  MKs = (Wq^T Wk)/32,
                vk = Wk^T bq          (S1[p,l] = sum_e H[e,p] x[e,l])
    G[e,p]    = MQs-contract xp + vq[e],         MQs = (Wq^T Wq)/32
    hq[p]     = (xp^T (Wq^T bq))/32 + |bq|^2     (S2T[p,l] = G-part + hq)
    E1        = exp(S1 - 10)  bf16; rowsum via activation accumulator
    M1[p,e]   = E1 @ x.T   (E1 transposed by the DMA xbar engine)
    cbv[p,d]  = (M1 @ Wv^T) / rowsum1 + bv
    E2        = exp(S2T - 40) fp16 (unnormalized)
    out[l,d]  = (sum_p E2[p,l] cbv[p,d]) / colsum2[l]
                -- contraction over p needs NO transpose (p is already on
                partitions); colsum2 rides along as N=2 ones-matmuls and is
                applied as a per-partition scale on the output copies.
    Output written [L, D] fp16; host transposes/upcasts to [D, L] f32.

Softmaxes use constant logit shifts (|S1|~21, |S2|~42) instead of max
subtraction; exp stays in range (bf16 for E1, fp16 for E2).
"""

import numpy as np
import ml_dtypes

import concourse.bass as bass
import concourse.mybir as mybir
import concourse.tile as tile
from concourse import bacc
from concourse.bass_utils import run_bass_kernel_spmd

F32 = mybir.dt.float32
F16 = mybir.dt.float16
BF16 = mybir.dt.bfloat16

B, D, L, P = 8, 512, 4096, 128
KT = D // 128      # 4 contraction tiles of 128
NCH = L // 512     # 8 l-chunks of 512
NLT = L // 128     # 32 l-tiles of 128
SEG = L // P       # 32: pool segment length
SHIFT1 = 10.0
SHIFT2 = 40.0

_CACHE = {}


def build():
    nc = bacc.Bacc(target_bir_lowering=False, trn_type="TRN2")
    X = nc.dram_tensor("x", [D, L], F16, kind="ExternalInput")
    MKS = nc.dram_tensor("mks", [D, D], F16, kind="ExternalInput")   # (Wq^T Wk)/32 [c,e]
    MQS = nc.dram_tensor("mqs", [D, D], F16, kind="ExternalInput")   # (Wq^T Wq)/32 [c,e]
    WVT = nc.dram_tensor("wvt", [D, D], BF16, kind="ExternalInput")  # Wv^T [e,d]
    IDN = nc.dram_tensor("ident", [128, 128], F16, kind="ExternalInput")
    INDS = nc.dram_tensor("inds", [128, 4], BF16, kind="ExternalInput")  # l -> l//32 one-hot
    ONES2 = nc.dram_tensor("ones2", [128, 2], BF16, kind="ExternalInput")
    VK = nc.dram_tensor("vk", [D], F32, kind="ExternalInput")        # Wk^T bq
    VQ = nc.dram_tensor("vq", [D], F32, kind="ExternalInput")        # Wq^T bq
    VQ2 = nc.dram_tensor("vq2", [D, 2], F16, kind="ExternalInput")   # [(Wq^T bq)/32, 0]
    BCONST = nc.dram_tensor("bconst", [128], F32, kind="ExternalInput")  # |bq|^2-SHIFT2
    BVB = nc.dram_tensor("bvb", [128, D], BF16, kind="ExternalInput")     # bv bcast
    OUT = nc.dram_tensor("out", [L, D], F16, kind="ExternalOutput")

    from contextlib import ExitStack
    with nc.allow_low_precision("16-bit matmul operands"), \
         tile.TileContext(nc) as tc, ExitStack() as stack:
        sb = stack.enter_context(tc.tile_pool(name="sb", bufs=1))
        e1p = stack.enter_context(tc.tile_pool(name="e1p", bufs=3))
        e1tp = stack.enter_context(tc.tile_pool(name="e1tp", bufs=3))
        e2p = stack.enter_context(tc.tile_pool(name="e2p", bufs=3))
        outp = stack.enter_context(tc.tile_pool(name="outp", bufs=2))
        iv2p = stack.enter_context(tc.tile_pool(name="iv2p", bufs=2))
        # PSUM (8 banks): xT 2 + xp 1 + hg 1 | pass1: s1 2 + m1 1 |
        # pass2: s2 2 + out 3 + rs2 1
        psT = stack.enter_context(tc.tile_pool(name="psT", bufs=2, space="PSUM"))
        psS = stack.enter_context(tc.tile_pool(name="psS", bufs=2, space="PSUM"))
        psM = stack.enter_context(tc.tile_pool(name="psM", bufs=1, space="PSUM"))
        psO = stack.enter_context(tc.tile_pool(name="psO", bufs=2, space="PSUM"))
        psR = stack.enter_context(tc.tile_pool(name="psR", bufs=1, space="PSUM"))

        # ---- ACT table warmup ------------------------------------------------
        warm = sb.tile([128, 1], F32)
        nc.vector.memset(warm, 0.0)
        nc.scalar.activation(out=warm, in_=warm,
                             func=mybir.ActivationFunctionType.Exp,
                             bias=warm, scale=1.0)

        # ---- input DMAs (order = DMA_ENGINES order) --------------------------
        ident = sb.tile([128, 128], F16)
        nc.gpsimd.dma_start(out=ident, in_=IDN[:, :])        # SWDGE, off HWDGE
        inds = sb.tile([128, 4], BF16)
        nc.gpsimd.dma_start(out=inds, in_=INDS[:, :])
        x_sb = sb.tile([128, KT, L], F16)
        xr = X.rearrange("(k p) l -> p k l", p=128)
        for ch in range(NCH):
            nc.sync.dma_start(out=x_sb[:, :, bass.ts(ch, 512)],
                              in_=xr[:, :, bass.ts(ch, 512)])
        mks = sb.tile([128, KT, D], F16)
        nc.sync.dma_start(out=mks, in_=MKS.rearrange("(k p) e -> p k e", p=128))
        vk = sb.tile([128, KT], F32)
        nc.gpsimd.dma_start(out=vk, in_=VK.rearrange("(k p) -> p k", p=128))
        vq = sb.tile([128, KT], F32)
        nc.gpsimd.dma_start(out=vq, in_=VQ.rearrange("(k p) -> p k", p=128))
        vq2 = sb.tile([128, KT, 2], F16)
        nc.gpsimd.dma_start(out=vq2, in_=VQ2.rearrange("(k p) t -> p k t", p=128))
        bconst = sb.tile([128, 1], F32)
        nc.gpsimd.dma_start(out=bconst, in_=BCONST.rearrange("(p o) -> p o", o=1))
        ones2 = sb.tile([128, 2], BF16)
        nc.gpsimd.dma_start(out=ones2, in_=ONES2[:, :])
        bvb = sb.tile([128, D], BF16)
        nc.gpsimd.dma_start(out=bvb, in_=BVB[:, :])
        mqs = sb.tile([128, KT, D], F16)
        nc.sync.dma_start(out=mqs, in_=MQS.rearrange("(k p) e -> p k e", p=128))
        wvt = sb.tile([128, KT, D], BF16)
        nc.sync.dma_start(out=wvt, in_=WVT.rearrange("(k p) e -> p k e", p=128))
        sh1 = sb.tile([128, 1], F32)
        nc.vector.memset(sh1, -SHIFT1)

        # ---- x.T tiles (PE transpose) + pooling (tiny PE matmuls) ------------
        xt = sb.tile([128, NLT, D], BF16)
        xpps = psM.tile([128, KT, 128], F32, tag="m")

        def pool_mm(jp):
            for h in range(2):
                j = 2 * jp + h
                for t in range(KT):
                    nc.tensor.matmul(xpps[:, t, 4 * j:4 * j + 4],
                                     xt[:, j, bass.ts(t, 128)], inds,
                                     start=True, stop=True)

        for jp in range(NLT // 2):
            tp = psT.tile([128, 2, 512], F16, tag="t")
            for h in range(2):
                j = 2 * jp + h
                for k in range(KT):
                    nc.tensor.transpose(tp[:, h, bass.ts(k, 128)],
                                        x_sb[:, k, bass.ts(j, 128)], ident)
            if jp % 3 == 0:
                nc.scalar.copy(xt[:, 2 * jp:2 * jp + 2, :], tp)
            else:
                nc.vector.tensor_copy(xt[:, 2 * jp:2 * jp + 2, :], tp)
            if jp >= 2:
                pool_mm(jp - 2)
        pool_mm(NLT // 2 - 2)
        pool_mm(NLT // 2 - 1)
        xp = sb.tile([128, KT, 128], F16)
        nc.vector.tensor_copy(xp, xpps)

        # ---- H, G, hq (copies on DVE to keep Act free) -----------------------
        h_sb = sb.tile([128, KT, 128], F16)
        hps = psR.tile([128, KT, 128], F32, tag="r")
        for et in range(KT):
            for ck in range(KT):
                nc.tensor.matmul(hps[:, et, :],
                                 mks[:, ck, bass.ts(et, 128)], xp[:, ck, :],
                                 start=(ck == 0), stop=(ck == KT - 1))
        for et in range(KT):
            nc.vector.tensor_scalar_add(h_sb[:, et, :], hps[:, et, :],
                                        vk[:, et:et + 1])
        hqps = psR.tile([128, 2], F32, tag="r")
        for ck in range(KT):
            nc.tensor.matmul(hqps, xp[:, ck, :], vq2[:, ck, :],
                             start=(ck == 0), stop=(ck == KT - 1))
        hq = sb.tile([128, 1], F32)
        nc.vector.tensor_scalar_add(hq, hqps[:, 0:1], bconst)
        g_sb = sb.tile([128, KT, 128], F16)
        gps = psR.tile([128, KT, 128], F32, tag="r")
        for et in range(KT):
            for ck in range(KT):
                nc.tensor.matmul(gps[:, et, :],
                                 mqs[:, ck, bass.ts(et, 128)], xp[:, ck, :],
                                 start=(ck == 0), stop=(ck == KT - 1))
        for et in range(KT):
            nc.vector.tensor_scalar_add(g_sb[:, et, :], gps[:, et, :],
                                        vq[:, et:et + 1])

        # ---- pass 1: S1 -> E1 -> (xbar) E1T -> M1, software-pipelined --------
        # M1 for chunk a is issued after S1 for chunk a+2, so the PE never
        # stalls on the exp + xbar-transpose round trip.
        rs1 = sb.tile([128, NCH], F32)
        m1ps2 = psM.tile([128, D], F32, tag="m")
        e1ts = []

        def s1_stage(a):
            s1 = psS.tile([128, 512], F32, tag="s")
            for k in range(KT):
                nc.tensor.matmul(s1, h_sb[:, k, :], x_sb[:, k, bass.ts(a, 512)],
                                 start=(k == 0), stop=(k == KT - 1))
            e1 = e1p.tile([128, 512], BF16, tag="e1")
            nc.scalar.activation(out=e1, in_=s1,
                                 func=mybir.ActivationFunctionType.Exp,
                                 bias=sh1, scale=1.0,
                                 accum_out=rs1[:, a:a + 1])
            e1t = e1tp.tile([128, 4, 128], BF16, tag="e1t")
            nc.sync.dma_start_transpose(e1t, e1)
            e1ts.append(e1t)

        def m1_stage(a):
            e1t = e1ts[a]
            for u in range(4):
                j = 4 * a + u
                nc.tensor.matmul(m1ps2, e1t[:, u, :], xt[:, j, :],
                                 start=(j == 0), stop=(j == NLT - 1))

        s1_stage(0)
        s1_stage(1)
        for a in range(NCH):
            if a + 2 < NCH:
                s1_stage(a + 2)
            m1_stage(a)

        rsum1 = sb.tile([128, 1], F32)
        nc.vector.reduce_sum(out=rsum1, in_=rs1, axis=mybir.AxisListType.X)
        inv1 = sb.tile([128, 1], F32)
        nc.vector.reciprocal(inv1, rsum1)

        # ---- c = (M1 @ Wv^T)/rowsum1 + bv ------------------------------------
        identb = sb.tile([128, 128], BF16)
        nc.vector.tensor_copy(identb, ident)
        m1 = sb.tile([128, D], BF16)
        nc.scalar.copy(m1, m1ps2)
        mtp = psT.tile([128, 2, 512], BF16, tag="t")
        for i in range(KT):
            nc.tensor.transpose(mtp[:, 0, bass.ts(i, 128)],
                                m1[:, bass.ts(i, 128)], identb)
        m1t = sb.tile([128, KT, 128], BF16)
        nc.scalar.copy(m1t, mtp[:, 0, :])
        cps = psR.tile([128, D], F32, tag="r")
        for i in range(KT):
            nc.tensor.matmul(cps, m1t[:, i, :], wvt[:, i, :],
                             start=(i == 0), stop=(i == KT - 1))
        cbv = sb.tile([128, D], BF16)
        nc.vector.scalar_tensor_tensor(out=cbv, in0=cps, scalar=inv1, in1=bvb,
                                       op0=mybir.AluOpType.mult,
                                       op1=mybir.AluOpType.add)

        # ---- pass 2: S2 -> E2 -> out = (E2^T cbv) * inv2 ---------------------
        # ident (fp16) reused as transpose stationary; e2 slices are the
        # stationary lhs of the output matmuls (contraction over p needs no
        # transpose), ones2 column rides along for colsum2.
        or_ = OUT.rearrange("(c j p) d -> c p j d", j=4, p=128)
        e2s = []

        def s2_stage(a):
            s2 = psS.tile([128, 512], F32, tag="s")
            for k in range(KT):
                nc.tensor.matmul(s2, g_sb[:, k, :], x_sb[:, k, bass.ts(a, 512)],
                                 start=(k == 0), stop=(k == KT - 1))
            e2 = e2p.tile([128, 512], BF16, tag="e2")
            nc.scalar.activation(out=e2, in_=s2,
                                 func=mybir.ActivationFunctionType.Exp,
                                 bias=hq, scale=1.0)
            e2s.append(e2)

        def out_stage(a):
            e2 = e2s[a]
            rsps = psR.tile([128, 4, 2], F32, tag="r")
            for u in range(4):
                nc.tensor.matmul(rsps[:, u, :], e2[:, bass.ts(u, 128)], ones2,
                                 start=True, stop=True)
            inv2 = iv2p.tile([128, 4], F32, tag="iv2")
            nc.vector.reciprocal(inv2, rsps[:, :, 0])
            o_sb = outp.tile([128, 4, D], F16, tag="o")
            for u in range(4):
                ops = psO.tile([128, D], F32, tag="o")
                nc.tensor.matmul(ops, e2[:, bass.ts(u, 128)], cbv,
                                 start=True, stop=True)
                if u % 2 == 0:
                    nc.scalar.activation(
                        out=o_sb[:, u, :], in_=ops,
                        func=mybir.ActivationFunctionType.Identity,
                        bias=0.0, scale=inv2[:, u:u + 1])
                else:
                    nc.vector.tensor_scalar_mul(o_sb[:, u, :], ops,
                                                inv2[:, u:u + 1])
            nc.sync.dma_start(out=or_[a], in_=o_sb)

        s2_stage(0)
        s2_stage(1)
        for a in range(NCH):
            if a + 2 < NCH:
                s2_stage(a + 2)
            out_stage(a)

    nc.compile()
    return nc


def _host_inputs(x, Wq, bq, Wk, bk, Wv, bv):
    del bk  # stage-1 softmax is invariant to the k-projection bias
    Wq = np.asarray(Wq, dtype=np.float32)
    Wk = np.asarray(Wk, dtype=np.float32)
    Wv = np.asarray(Wv, dtype=np.float32)
    bq = np.asarray(bq, dtype=np.float32)
    bv = np.asarray(bv, dtype=np.float32)
    bf16 = ml_dtypes.bfloat16
    inds = np.zeros((128, 4), dtype=np.float32)
    inds[np.arange(128), np.arange(128) // SEG] = 1.0
    ones2 = np.zeros((128, 2), dtype=np.float32)
    ones2[:, 0] = 1.0
    common = {
        "mks": ((Wq.T @ Wk) / SEG).astype(np.float16),
        "mqs": ((Wq.T @ Wq) / SEG).astype(np.float16),
        "wvt": np.ascontiguousarray(Wv.T).astype(bf16),
        "ident": np.eye(128, dtype=np.float16),
        "inds": inds.astype(bf16),
        "ones2": ones2.astype(bf16),
        "vk": (Wk.T @ bq).astype(np.float32),
        "vq": (Wq.T @ bq).astype(np.float32),
        "vq2": np.stack([(Wq.T @ bq) / SEG, np.zeros(D, np.float32)],
                        axis=1).astype(np.float16),
        "bconst": np.full(128, float(bq @ bq) - SHIFT2, dtype=np.float32),
        "bvb": np.tile(bv[None, :], (128, 1)).astype(bf16),
    }
    maps = []
    for b in range(B):
        m = dict(common)
        m["x"] = np.ascontiguousarray(x[b]).astype(np.float16)
        maps.append(m)
    return maps


def kernel(x, Wq, bq, Wk, bk, Wv, bv):
    x = np.asarray(x, dtype=np.float32)
    if "nc" not in _CACHE:
        _CACHE["nc"] = build()
    nc = _CACHE["nc"]
    in_maps = _host_inputs(x, Wq, bq, Wk, bk, Wv, bv)
    res = run_bass_kernel_spmd(nc, in_maps, core_ids=list(range(B)))
    out = np.empty((B, D, L), dtype=np.float32)
    for b in range(B):
        out[b] = np.asarray(res.results[b]["out"]).astype(np.float32).T
    return out


# revision 21
# speedup vs baseline: 1.4264x; 1.0476x over previous
"""AgentSelfAttention1d Trainium2 kernel (v2).

Per batch b (one NeuronCore each):
    xt = x[b].T                       # [L=4096, D=512]
    q/k/v = xt @ W{q,k,v}.T + b       # [L, D]
    a  = AdaptiveAvgPool(q) -> [P=128, D]
    c  = softmax(a @ k.T, -1) @ v     # [P, D]
    r  = softmax(q @ a.T, -1) @ c     # [L, D]
    out[b] = r.T                      # [D, L]

Restructuring (all projections folded into host-precomputed weight
products; everything channel-first on chip):
    xp[c,p]   = seg-sum of x over 32-wide windows      (via tiny PE matmuls
                against a one-hot segment indicator, from the x.T tiles)
    H[e,p]    = MKs[c,e]^T-contract xp + vk[e],  MKs = (Wq^T Wk)/32,
                vk = Wk^T bq          (S1[p,l] = sum_e H[e,p] x[e,l])
    G[e,p]    = MQs-contract xp + vq[e],         MQs = (Wq^T Wq)/32
    hq[p]     = (xp^T (Wq^T bq))/32 + |bq|^2     (S2T[p,l] = G-part + hq)
    E1        = exp(S1 - 10)  bf16; rowsum via activation accumulator
    M1[p,e]   = E1 @ x.T   (E1 transposed by the DMA xbar engine)
    cbv[p,d]  = (M1 @ Wv^T) / rowsum1 + bv
    E2        = exp(S2T - 40) fp16 (unnormalized)
    out[l,d]  = (sum_p E2[p,l] cbv[p,d]) / colsum2[l]
                -- contraction over p needs NO transpose (p is already on
                partitions); colsum2 rides along as N=2 ones-matmuls and is
                applied as a per-partition scale on the output copies.
    Output written [L, D] fp16; host transposes/upcasts to [D, L] f32.

Softmaxes use constant logit shifts (|S1|~21, |S2|~42) instead of max
subtraction; exp stays in range (bf16 for E1, fp16 for E2).
"""

import numpy as np
import ml_dtypes

import concourse.bass as bass
import concourse.mybir as mybir
import concourse.tile as tile
from concourse import bacc
from concourse.bass_utils import run_bass_kernel_spmd

F32 = mybir.dt.float32
F16 = mybir.dt.float16
BF16 = mybir.dt.bfloat16

B, D, L, P = 8, 512, 4096, 128
KT = D // 128      # 4 contraction tiles of 128
NCH = L // 512     # 8 l-chunks of 512
NLT = L // 128     # 32 l-tiles of 128
SEG = L // P       # 32: pool segment length
SHIFT1 = 10.0
SHIFT2 = 40.0

_CACHE = {}


def build():
    nc = bacc.Bacc(target_bir_lowering=False, trn_type="TRN2")
    X = nc.dram_tensor("x", [D, L], F16, kind="ExternalInput")
    MKS = nc.dram_tensor("mks", [D, D], F16, kind="ExternalInput")   # (Wq^T Wk)/32 [c,e]
    MQS = nc.dram_tensor("mqs", [D, D], F16, kind="ExternalInput")   # (Wq^T Wq)/32 [c,e]
    WVT = nc.dram_tensor("wvt", [D, D], BF16, kind="ExternalInput")  # Wv^T [e,d]
    IDN = nc.dram_tensor("ident", [128, 128], F16, kind="ExternalInput")
    INDS = nc.dram_tensor("inds", [128, 4], BF16, kind="ExternalInput")  # l -> l//32 one-hot
    ONES2 = nc.dram_tensor("ones2", [128, 2], BF16, kind="ExternalInput")
    VK = nc.dram_tensor("vk", [D], F32, kind="ExternalInput")        # Wk^T bq
    VQ = nc.dram_tensor("vq", [D], F32, kind="ExternalInput")        # Wq^T bq
    VQ2 = nc.dram_tensor("vq2", [D, 2], F16, kind="ExternalInput")   # [(Wq^T bq)/32, 0]
    BCONST = nc.dram_tensor("bconst", [128], F32, kind="ExternalInput")  # |bq|^2-SHIFT2
    BVB = nc.dram_tensor("bvb", [128, D], BF16, kind="ExternalInput")     # bv bcast
    OUT = nc.dram_tensor("out", [L, D], F16, kind="ExternalOutput")

    from contextlib import ExitStack
    with nc.allow_low_precision("16-bit matmul operands"), \
         tile.TileContext(nc) as tc, ExitStack() as stack:
        sb = stack.enter_context(tc.tile_pool(name="sb", bufs=1))
        e1p = stack.enter_context(tc.tile_pool(name="e1p", bufs=3))
        e1tp = stack.enter_context(tc.tile_pool(name="e1tp", bufs=3))
        e2p = stack.enter_context(tc.tile_pool(name="e2p", bufs=3))
        outp = stack.enter_context(tc.tile_pool(name="outp", bufs=2))
        iv2p = stack.enter_context(tc.tile_pool(name="iv2p", bufs=2))
        # PSUM (8 banks): xT 2 + xp 1 + hg 1 | pass1: s1 2 + m1 1 |
        # pass2: s2 2 + out 3 + rs2 1
        psT = stack.enter_context(tc.tile_pool(name="psT", bufs=2, space="PSUM"))
        psS = stack.enter_context(tc.tile_pool(name="psS", bufs=2, space="PSUM"))
        psM = stack.enter_context(tc.tile_pool(name="psM", bufs=1, space="PSUM"))
        psO = stack.enter_context(tc.tile_pool(name="psO", bufs=2, space="PSUM"))
        psR = stack.enter_context(tc.tile_pool(name="psR", bufs=1, space="PSUM"))

        # ---- ACT table warmup ------------------------------------------------
        warm = sb.tile([128, 1], F32)
        nc.vector.memset(warm, 0.0)
        nc.scalar.activation(out=warm, in_=warm,
                             func=mybir.ActivationFunctionType.Exp,
                             bias=warm, scale=1.0)

        # ---- input DMAs (order = DMA_ENGINES order) --------------------------
        ident = sb.tile([128, 128], F16)
        nc.gpsimd.dma_start(out=ident, in_=IDN[:, :])        # SWDGE, off HWDGE
        inds = sb.tile([128, 4], BF16)
        nc.gpsimd.dma_start(out=inds, in_=INDS[:, :])
        x_sb = sb.tile([128, KT, L], F16)
        xr = X.rearrange("(k p) l -> p k l", p=128)
        for ch in range(NCH):
            nc.sync.dma_start(out=x_sb[:, :, bass.ts(ch, 512)],
                              in_=xr[:, :, bass.ts(ch, 512)])
        mks = sb.tile([128, KT, D], F16)
        nc.sync.dma_start(out=mks, in_=MKS.rearrange("(k p) e -> p k e", p=128))
        vk = sb.tile([128, KT], F32)
        nc.gpsimd.dma_start(out=vk, in_=VK.rearrange("(k p) -> p k", p=128))
        vq = sb.tile([128, KT], F32)
        nc.gpsimd.dma_start(out=vq, in_=VQ.rearrange("(k p) -> p k", p=128))
        vq2 = sb.tile([128, KT, 2], F16)
        nc.gpsimd.dma_start(out=vq2, in_=VQ2.rearrange("(k p) t -> p k t", p=128))
        bconst = sb.tile([128, 1], F32)
        nc.gpsimd.dma_start(out=bconst, in_=BCONST.rearrange("(p o) -> p o", o=1))
        ones2 = sb.tile([128, 2], BF16)
        nc.gpsimd.dma_start(out=ones2, in_=ONES2[:, :])
        bvb = sb.tile([128, D], BF16)
        nc.gpsimd.dma_start(out=bvb, in_=BVB[:, :])
        mqs = sb.tile([128, KT, D], F16)
        nc.sync.dma_start(out=mqs, in_=MQS.rearrange("(k p) e -> p k e", p=128))
        wvt = sb.tile([128, KT, D], BF16)
        nc.sync.dma_start(out=wvt, in_=WVT.rearrange("(k p) e -> p k e", p=128))
        sh1 = sb.tile([128, 1], F32)
        nc.vector.memset(sh1, -SHIFT1)

        # ---- x.T tiles (PE transpose) + pooling (tiny PE matmuls) ------------
        xt = sb.tile([128, NLT, D], BF16)
        xpps = psM.tile([128, KT, 128], F32, tag="m")

        def pool_mm(jp):
            for h in range(2):
                j = 2 * jp + h
                for t in range(KT):
                    nc.tensor.matmul(xpps[:, t, 4 * j:4 * j + 4],
                                     xt[:, j, bass.ts(t, 128)], inds,
                                     start=True, stop=True)

        for jp in range(NLT // 2):
            tp = psT.tile([128, 2, 512], F16, tag="t")
            for h in range(2):
                j = 2 * jp + h
                for k in range(KT):
                    nc.tensor.transpose(tp[:, h, bass.ts(k, 128)],
                                        x_sb[:, k, bass.ts(j, 128)], ident)
            if jp % 3 == 0:
                nc.scalar.copy(xt[:, 2 * jp:2 * jp + 2, :], tp)
            else:
                nc.vector.tensor_copy(xt[:, 2 * jp:2 * jp + 2, :], tp)
            if jp >= 2:
                pool_mm(jp - 2)
        pool_mm(NLT // 2 - 2)
        pool_mm(NLT // 2 - 1)
        xp = sb.tile([128, KT, 128], F16)
        nc.vector.tensor_copy(xp, xpps)

        # ---- H, G, hq (copies on DVE to keep Act free) -----------------------
        h_sb = sb.tile([128, KT, 128], F16)
        hps = psR.tile([128, KT, 128], F32, tag="r")
        for et in range(KT):
            for ck in range(KT):
                nc.tensor.matmul(hps[:, et, :],
                                 mks[:, ck, bass.ts(et, 128)], xp[:, ck, :],
                                 start=(ck == 0), stop=(ck == KT - 1))
        for et in range(KT):
            nc.vector.tensor_scalar_add(h_sb[:, et, :], hps[:, et, :],
                                        vk[:, et:et + 1])
        hqps = psR.tile([128, 2], F32, tag="r")
        for ck in range(KT):
            nc.tensor.matmul(hqps, xp[:, ck, :], vq2[:, ck, :],
                             start=(ck == 0), stop=(ck == KT - 1))
        hq = sb.tile([128, 1], F32)
        nc.vector.tensor_scalar_add(hq, hqps[:, 0:1], bconst)
        g_sb = sb.tile([128, KT, 128], F16)
        gps = psR.tile([128, KT, 128], F32, tag="r")
        for et in range(KT):
            for ck in range(KT):
                nc.tensor.matmul(gps[:, et, :],
                                 mqs[:, ck, bass.ts(et, 128)], xp[:, ck, :],
                                 start=(ck == 0), stop=(ck == KT - 1))
        for et in range(KT):
            nc.vector.tensor_scalar_add(g_sb[:, et, :], gps[:, et, :],
                                        vq[:, et:et + 1])

        # ---- pass 1: S1 -> E1 -> (xbar) E1T -> M1, software-pipelined --------
        # M1 for chunk a is issued after S1 for chunk a+2, so the PE never
        # stalls on the exp + xbar-transpose round trip.
        rs1 = sb.tile([128, NCH], F32)
        m1ps2 = psM.tile([128, D], F32, tag="m")
        e1ts = []

        def s1_stage(a):
            s1 = psS.tile([128, 512], F32, tag="s")
            for k in range(KT):
                nc.tensor.matmul(s1, h_sb[:, k, :], x_sb[:, k, bass.ts(a, 512)],
                                 start=(k == 0), stop=(k == KT - 1))
            e1 = e1p.tile([128, 512], BF16, tag="e1")
            nc.scalar.activation(out=e1, in_=s1,
                                 func=mybir.ActivationFunctionType.Exp,
                                 bias=sh1, scale=1.0,
                                 accum_out=rs1[:, a:a + 1])
            e1t = e1tp.tile([128, 4, 128], BF16, tag="e1t")
            nc.sync.dma_start_transpose(e1t, e1)
            e1ts.append(e1t)

        def m1_stage(a):
            e1t = e1ts[a]
            for u in range(4):
                j = 4 * a + u
                nc.tensor.matmul(m1ps2, e1t[:, u, :], xt[:, j, :],
                                 start=(j == 0), stop=(j == NLT - 1))

        s1_stage(0)
        s1_stage(1)
        for a in range(NCH):
            if a + 2 < NCH:
                s1_stage(a + 2)
            m1_stage(a)

        rsum1 = sb.tile([128, 1], F32)
        nc.vector.reduce_sum(out=rsum1, in_=rs1, axis=mybir.AxisListType.X)
        inv1 = sb.tile([128, 1], F32)
        nc.vector.reciprocal(inv1, rsum1)

        # ---- c = (M1 @ Wv^T)/rowsum1 + bv ------------------------------------
        identb = sb.tile([128, 128], BF16)
        nc.vector.tensor_copy(identb, ident)
        m1 = sb.tile([128, D], BF16)
        nc.scalar.copy(m1, m1ps2)
        mtp = psT.tile([128, 2, 512], BF16, tag="t")
        for i in range(KT):
            nc.tensor.transpose(mtp[:, 0, bass.ts(i, 128)],
                                m1[:, bass.ts(i, 128)], identb)
        m1t = sb.tile([128, KT, 128], BF16)
        nc.scalar.copy(m1t, mtp[:, 0, :])
        cps = psR.tile([128, D], F32, tag="r")
        for i in range(KT):
            nc.tensor.matmul(cps, m1t[:, i, :], wvt[:, i, :],
                             start=(i == 0), stop=(i == KT - 1))
        cbv = sb.tile([128, D], BF16)
        nc.vector.scalar_tensor_tensor(out=cbv, in0=cps, scalar=inv1, in1=bvb,
                                       op0=mybir.AluOpType.mult,
                                       op1=mybir.AluOpType.add)

        # ---- pass 2: S2 -> E2 -> out = (E2^T cbv) * inv2 ---------------------
        # ident (fp16) reused as transpose stationary; e2 slices are the
        # stationary lhs of the output matmuls (contraction over p needs no
        # transpose), ones2 column rides along for colsum2.
        or_ = OUT.rearrange("(c j p) d -> c p j d", j=4, p=128)
        e2s = []

        def s2_stage(a):
            s2 = psS.tile([128, 512], F32, tag="s")
            for k in range(KT):
                nc.tensor.matmul(s2, g_sb[:, k, :], x_sb[:, k, bass.ts(a, 512)],
                                 start=(k == 0), stop=(k == KT - 1))
            e2 = e2p.tile([128, 512], BF16, tag="e2")
            nc.scalar.activation(out=e2, in_=s2,
                                 func=mybir.ActivationFunctionType.Exp,
                                 bias=hq, scale=1.0)
            e2s.append(e2)

        def out_stage(a):
            e2 = e2s[a]
            rsps = psR.tile([128, 4, 2], F32, tag="r")
            for u in range(4):
                nc.tensor.matmul(rsps[:, u, :], e2[:, bass.ts(u, 128)], ones2,
                                 start=True, stop=True)
            inv2 = iv2p.tile([128, 4], F32, tag="iv2")
            nc.vector.reciprocal(inv2, rsps[:, :, 0])
            o_sb = outp.tile([128, 4, D], F16, tag="o")
            for u in range(4):
                if u < 2:
                    ops = psO.tile([128, D], F32, tag="o")
                else:
                    ops = psT.tile([128, 512], F32, tag="t")
                nc.tensor.matmul(ops, e2[:, bass.ts(u, 128)], cbv,
                                 start=True, stop=True)
                if u % 2 == 0:
                    nc.scalar.activation(
                        out=o_sb[:, u, :], in_=ops,
                        func=mybir.ActivationFunctionType.Identity,
                        bias=0.0, scale=inv2[:, u:u + 1])
                else:
                    nc.vector.tensor_scalar_mul(o_sb[:, u, :], ops,
                                                inv2[:, u:u + 1])
            nc.sync.dma_start(out=or_[a], in_=o_sb)

        s2_stage(0)
        s2_stage(1)
        for a in range(NCH):
            if a + 2 < NCH:
                s2_stage(a + 2)
            out_stage(a)

    nc.compile()
    return nc


def _host_inputs(x, Wq, bq, Wk, bk, Wv, bv):
    del bk  # stage-1 softmax is invariant to the k-projection bias
    Wq = np.asarray(Wq, dtype=np.float32)
    Wk = np.asarray(Wk, dtype=np.float32)
    Wv = np.asarray(Wv, dtype=np.float32)
    bq = np.asarray(bq, dtype=np.float32)
    bv = np.asarray(bv, dtype=np.float32)
    bf16 = ml_dtypes.bfloat16
    inds = np.zeros((128, 4), dtype=np.float32)
    inds[np.arange(128), np.arange(128) // SEG] = 1.0
    ones2 = np.zeros((128, 2), dtype=np.float32)
    ones2[:, 0] = 1.0
    common = {
        "mks": ((Wq.T @ Wk) / SEG).astype(np.float16),
        "mqs": ((Wq.T @ Wq) / SEG).astype(np.float16),
        "wvt": np.ascontiguousarray(Wv.T).astype(bf16),
        "ident": np.eye(128, dtype=np.float16),
        "inds": inds.astype(bf16),
        "ones2": ones2.astype(bf16),
        "vk": (Wk.T @ bq).astype(np.float32),
        "vq": (Wq.T @ bq).astype(np.float32),
        "vq2": np.stack([(Wq.T @ bq) / SEG, np.zeros(D, np.float32)],
                        axis=1).astype(np.float16),
        "bconst": np.full(128, float(bq @ bq) - SHIFT2, dtype=np.float32),
        "bvb": np.tile(bv[None, :], (128, 1)).astype(bf16),
    }
    maps = []
    for b in range(B):
        m = dict(common)
        m["x"] = np.ascontiguousarray(x[b]).astype(np.float16)
        maps.append(m)
    return maps


def kernel(x, Wq, bq, Wk, bk, Wv, bv):
    x = np.asarray(x, dtype=np.float32)
    if "nc" not in _CACHE:
        _CACHE["nc"] = build()
    nc = _CACHE["nc"]
    in_maps = _host_inputs(x, Wq, bq, Wk, bk, Wv, bv)
    res = run_bass_kernel_spmd(nc, in_maps, core_ids=list(range(B)))
    out = np.empty((B, D, L), dtype=np.float32)
    for b in range(B):
        out[b] = np.asarray(res.results[b]["out"]).astype(np.float32).T
    return out
